# revision 9
# baseline (speedup 1.0000x reference)
"""Lie-series expansion kernel for Trainium2 (8 NeuronCores, data-parallel).

result = x + sum_{i=1..order} z_i,  z_i = (1/i) * sum_g diag(theta_g) z_{i-1} A_g

Per step the G=8 generator contraction fuses into ONE [B,4096]x[4096,512]
matmul: stack W_(g,f) = (theta_g/i) * z_{i-1} along the contraction dim.
Data-parallel over batch: each core owns B/8=512 rows, keeps z TRANSPOSED
([feature_partitions, batch_free]) so the theta scaling is a DVE
elementwise op and algebra A[g,f,h] is the stationary operand in natural
layout. Everything lives in SBUF across all steps; float32r matmuls run
at 1 cycle/row (4x over plain fp32).
"""

import numpy as np

import concourse.bass as bass
import concourse.bacc as bacc
import concourse.mybir as mybir
from concourse import tile
from concourse.bass_utils import run_bass_kernel_spmd

G, B, F = 8, 4096, 512
NCORES = 8
BLOC = B // NCORES          # 512 batch rows per core
P = 128                     # partitions
FT = F // P                 # 4 feature tiles
NK = G * FT                 # 32 contraction k-tiles per step
DT = mybir.dt.float32
DTR = mybir.dt.float32r
MULT = mybir.AluOpType.mult

_cache = {}


def _build(order: int):
    if order in _cache:
        return _cache[order]

    nc = bacc.Bacc("TRN2", target_bir_lowering=False, debug=False,
                   num_devices=NCORES)

    A_d = nc.dram_tensor("A", [P, NK * F], DTR, kind="ExternalInput")
    th_d = nc.dram_tensor("th", [P, G * BLOC], DT, kind="ExternalInput")
    xT_d = nc.dram_tensor("xT", [P, FT * BLOC], DT, kind="ExternalInput")
    out_d = nc.dram_tensor("outT", [F, BLOC], DT, kind="ExternalOutput")

    # Consumption order of contraction k-tiles: fi-major (fi outer, g inner).
    korder = [g * FT + fi for fi in range(FT) for g in range(G)]

    with tile.TileContext(nc) as tc:
        with (
            tc.tile_pool(name="const", bufs=1) as cpool,
            tc.tile_pool(name="z", bufs=2) as zpool,
            tc.tile_pool(name="w", bufs=2) as wpool,
            tc.tile_pool(name="psum", bufs=2, space=bass.MemorySpace.PSUM) as ppool,
        ):
            # Two parallel HWDGE rings (sync + scalar). x^T and theta land as
            # ONE big DMA each (a single InstDMACopy fans out across all 16
            # SDMA engines); A k-tiles then stream in step-1 consumption
            # order so the PE chases the DMA.
            rings = [nc.sync, nc.scalar]
            xT0 = cpool.tile([P, FT * BLOC], DT, tag="xT0", name="xT0")
            nc.sync.dma_start(xT0[:], xT_d[:])
            th_t = cpool.tile([P, G * BLOC], DT, tag="th", name="th")
            nc.scalar.dma_start(th_t[:], th_d[:])
            th = [th_t[:, g * BLOC:(g + 1) * BLOC] for g in range(G)]
            zT = [xT0[:, ft * BLOC:(ft + 1) * BLOC] for ft in range(FT)]
            res = []
            for ft in range(FT):
                rt = cpool.tile([P, BLOC], DT, tag=f"res{ft}", name=f"res{ft}")
                nc.vector.tensor_copy(rt[:], zT[ft][:])
                res.append(rt)

            A_t = [None] * NK
            for n, k in enumerate(korder):
                at = cpool.tile([P, F], DTR, tag=f"A{k}", name=f"A{k}")
                rings[n % 2].dma_start(at[:], A_d[:, k * F:(k + 1) * F])
                A_t[k] = at

            def w_build(i, k, src):
                g = k // FT
                w = wpool.tile([P, BLOC], DTR, tag="w", bufs=40,
                               name=f"w_{i}_{k}")
                # w = (theta_g / i) * z_{i-1}
                nc.vector.scalar_tensor_tensor(
                    w[:], th[g][:, :], 1.0 / i, src[:], MULT, MULT)
                return w

            def drain(i, ps_ho, ho, Wn):
                """Consume step i's completed psum bank `ho`: either fold into
                res (last step) or copy to SBUF, accumulate, and build step
                i+1's W tiles for fi=ho."""
                if i == order:
                    nc.vector.scalar_tensor_tensor(
                        res[ho][:], ps_ho[:], 1.0, res[ho][:],
                        MULT, mybir.AluOpType.add)
                    nc.sync.dma_start(out_d[ho * P:(ho + 1) * P, :],
                                      res[ho][:])
                else:
                    zt = zpool.tile([P, BLOC], DT, tag=f"z{ho}",
                                    name=f"z{ho}_{i}")
                    nc.scalar.copy(zt[:], ps_ho[:])
                    nc.vector.tensor_add(res[ho][:], res[ho][:], zt[:])
                    for g in range(G):
                        Wn[g * FT + ho] = w_build(i + 1, g * FT + ho, zt)

            # ---- step 1: fi-outer, W built inline from x^T (chases DMA) ----
            W = [None] * NK
            ps = [ppool.tile([P, BLOC], DT, tag=f"ps{ho}", name=f"ps{ho}_1")
                  for ho in range(FT)]
            for n, k in enumerate(korder):
                W[k] = w_build(1, k, zT[k % FT])
                for ho in range(FT):
                    nc.tensor.matmul(
                        ps[ho][:], A_t[k][:, ho * P:(ho + 1) * P], W[k][:],
                        start=(n == 0), stop=(n == NK - 1))
            Wn = [None] * NK
            for ho in range(FT):
                drain(1, ps[ho], ho, Wn)
            W = Wn

            # ---- steps 2..order: ho-outer so psum banks complete early and
            # step i+1's W tiles pre-build during step i (no boundary bubble)
            for i in range(2, order + 1):
                Wn = [None] * NK
                psn = [ppool.tile([P, BLOC], DT, tag=f"ps{ho}",
                                  name=f"ps{ho}_{i}") for ho in range(FT)]
                for ho in range(FT):
                    for n, k in enumerate(korder):
                        nc.tensor.matmul(
                            psn[ho][:], A_t[k][:, ho * P:(ho + 1) * P], W[k][:],
                            start=(n == 0), stop=(n == NK - 1))
                    drain(i, psn[ho], ho, Wn)
                W = Wn

    nc.compile()
    _cache[order] = nc
    return nc


def _in_maps(theta, x, algebra):
    theta = np.ascontiguousarray(theta, dtype=np.float32)
    x = np.ascontiguousarray(x, dtype=np.float32)
    algebra = np.ascontiguousarray(algebra, dtype=np.float32)
    # A[g, f, h] -> A_host[p, (g*FT+fi)*F + h] with f = fi*128 + p
    A_host = np.ascontiguousarray(
        algebra.reshape(G, FT, P, F).transpose(2, 0, 1, 3).reshape(P, NK * F))
    maps = []
    for c in range(NCORES):
        th_loc = theta[:, c * BLOC:(c + 1) * BLOC]          # [G, BLOC]
        th_b = np.ascontiguousarray(
            np.broadcast_to(th_loc[None], (P, G, BLOC)).reshape(P, G * BLOC))
        xT = np.ascontiguousarray(
            x[c * BLOC:(c + 1) * BLOC, :].T.reshape(FT, P, BLOC)
            .transpose(1, 0, 2).reshape(P, FT * BLOC))
        maps.append({"A": A_host, "th": th_b, "xT": xT})
    return maps


def _run(theta, x, algebra, order, **kw):
    nc = _build(int(order))
    res = run_bass_kernel_spmd(nc, _in_maps(theta, x, algebra),
                               list(range(NCORES)), **kw)
    out = np.empty((B, F), dtype=np.float32)
    for c in range(NCORES):
        out[c * BLOC:(c + 1) * BLOC, :] = res.results[c]["outT"].T
    return out, res


def kernel(theta, x, algebra, order):
    out, _ = _run(theta, x, algebra, order)
    return out


# revision 10
# speedup vs baseline: 1.0613x; 1.0613x over previous
"""Lie-series expansion kernel for Trainium2 (8 NeuronCores, data-parallel).

result = x + sum_{i=1..order} z_i,  z_i = (1/i) * sum_g diag(theta_g) z_{i-1} A_g

Per step the G=8 generator contraction fuses into ONE [B,4096]x[4096,512]
matmul: stack W_(g,f) = (theta_g/i) * z_{i-1} along the contraction dim.
Data-parallel over batch: each core owns B/8=512 rows, keeps z TRANSPOSED
([feature_partitions, batch_free]) so the theta scaling is a DVE
elementwise op and algebra A[g,f,h] is the stationary operand in natural
layout. Everything lives in SBUF across all steps; float32r matmuls run
at 1 cycle/row (4x over plain fp32).
"""

import numpy as np

import concourse.bass as bass
import concourse.bacc as bacc
import concourse.mybir as mybir
from concourse import tile
from concourse.bass_utils import run_bass_kernel_spmd

G, B, F = 8, 4096, 512
NCORES = 8
BLOC = B // NCORES          # 512 batch rows per core
P = 128                     # partitions
FT = F // P                 # 4 feature tiles
NK = G * FT                 # 32 contraction k-tiles per step
DT = mybir.dt.float32
DTR = mybir.dt.float32r
MULT = mybir.AluOpType.mult

_cache = {}


def _build(order: int):
    if order in _cache:
        return _cache[order]

    nc = bacc.Bacc("TRN2", target_bir_lowering=False, debug=False,
                   num_devices=NCORES)

    A_d = nc.dram_tensor("A", [P, NK * F], DTR, kind="ExternalInput")
    th_d = nc.dram_tensor("th", [P, G * BLOC], DT, kind="ExternalInput")
    xT_d = nc.dram_tensor("xT", [P, FT * BLOC], DT, kind="ExternalInput")
    out_d = nc.dram_tensor("outT", [F, BLOC], DT, kind="ExternalOutput")

    # Consumption order of contraction k-tiles: fi-major (fi outer, g inner).
    korder = [g * FT + fi for fi in range(FT) for g in range(G)]

    with tile.TileContext(nc) as tc:
        with (
            tc.tile_pool(name="const", bufs=1) as cpool,
            tc.tile_pool(name="z", bufs=2) as zpool,
            tc.tile_pool(name="w", bufs=2) as wpool,
            tc.tile_pool(name="psum", bufs=2, space=bass.MemorySpace.PSUM) as ppool,
        ):
            # Two parallel HWDGE rings (sync + scalar), fed round-robin with
            # 256KB chunks in step-1 TRUE dependency order: xT0, th0, A0,
            # th1, A4, ... so the first matmul unblocks after ~3 chunks and
            # the PE chases the stream with 3x supply margin.
            rings = [nc.sync, nc.scalar]
            th = [None] * G
            zT = [None] * FT
            A_t = [None] * NK

            loads = [("x", 0), ("t", 0), ("a", korder[0])]
            for g in range(1, G):
                loads += [("t", g), ("a", korder[g])]
            loads += [("x", 1)] + [("a", k) for k in korder[8:16]]
            loads += [("x", 2)] + [("a", k) for k in korder[16:24]]
            loads += [("x", 3)] + [("a", k) for k in korder[24:32]]

            for n, (kind, j) in enumerate(loads):
                ring = rings[n % 2]
                if kind == "x":
                    zT[j] = zpool.tile([P, BLOC], DT, tag=f"z{j}",
                                       name=f"z{j}_init")
                    ring.dma_start(zT[j][:],
                                   xT_d[:, j * BLOC:(j + 1) * BLOC])
                elif kind == "t":
                    th[j] = cpool.tile([P, BLOC], DT, tag=f"th{j}",
                                       name=f"th{j}")
                    ring.dma_start(th[j][:],
                                   th_d[:, j * BLOC:(j + 1) * BLOC])
                else:
                    A_t[j] = cpool.tile([P, F], DTR, tag=f"A{j}",
                                        name=f"A{j}")
                    ring.dma_start(A_t[j][:], A_d[:, j * F:(j + 1) * F])

            res = []
            for ft in range(FT):
                rt = cpool.tile([P, BLOC], DT, tag=f"res{ft}", name=f"res{ft}")
                nc.vector.tensor_copy(rt[:], zT[ft][:])
                res.append(rt)

            def w_build(i, k, src):
                g = k // FT
                w = wpool.tile([P, BLOC], DTR, tag="w", bufs=40,
                               name=f"w_{i}_{k}")
                # w = (theta_g / i) * z_{i-1}
                nc.vector.scalar_tensor_tensor(
                    w[:], th[g][:, :], 1.0 / i, src[:], MULT, MULT)
                return w

            def drain(i, ps_ho, ho, Wn):
                """Consume step i's completed psum bank `ho`: either fold into
                res (last step) or copy to SBUF, accumulate, and build step
                i+1's W tiles for fi=ho."""
                if i == order:
                    nc.vector.scalar_tensor_tensor(
                        res[ho][:], ps_ho[:], 1.0, res[ho][:],
                        MULT, mybir.AluOpType.add)
                    nc.sync.dma_start(out_d[ho * P:(ho + 1) * P, :],
                                      res[ho][:])
                else:
                    zt = zpool.tile([P, BLOC], DT, tag=f"z{ho}",
                                    name=f"z{ho}_{i}")
                    nc.scalar.copy(zt[:], ps_ho[:])
                    nc.vector.tensor_add(res[ho][:], res[ho][:], zt[:])
                    for g in range(G):
                        Wn[g * FT + ho] = w_build(i + 1, g * FT + ho, zt)

            # ---- step 1: fi-outer, W built inline from x^T (chases DMA) ----
            W = [None] * NK
            ps = [ppool.tile([P, BLOC], DT, tag=f"ps{ho}", name=f"ps{ho}_1")
                  for ho in range(FT)]
            for n, k in enumerate(korder):
                W[k] = w_build(1, k, zT[k % FT])
                for ho in range(FT):
                    nc.tensor.matmul(
                        ps[ho][:], A_t[k][:, ho * P:(ho + 1) * P], W[k][:],
                        start=(n == 0), stop=(n == NK - 1))
            Wn = [None] * NK
            for ho in range(FT):
                drain(1, ps[ho], ho, Wn)
            W = Wn

            # ---- steps 2..order: ho-outer so psum banks complete early and
            # step i+1's W tiles pre-build during step i (no boundary bubble)
            for i in range(2, order + 1):
                Wn = [None] * NK
                psn = [ppool.tile([P, BLOC], DT, tag=f"ps{ho}",
                                  name=f"ps{ho}_{i}") for ho in range(FT)]
                for ho in range(FT):
                    for n, k in enumerate(korder):
                        nc.tensor.matmul(
                            psn[ho][:], A_t[k][:, ho * P:(ho + 1) * P], W[k][:],
                            start=(n == 0), stop=(n == NK - 1))
                    drain(i, psn[ho], ho, Wn)
                W = Wn

    nc.compile()
    _cache[order] = nc
    return nc


def _in_maps(theta, x, algebra):
    theta = np.ascontiguousarray(theta, dtype=np.float32)
    x = np.ascontiguousarray(x, dtype=np.float32)
    algebra = np.ascontiguousarray(algebra, dtype=np.float32)
    # A[g, f, h] -> A_host[p, (g*FT+fi)*F + h] with f = fi*128 + p
    A_host = np.ascontiguousarray(
        algebra.reshape(G, FT, P, F).transpose(2, 0, 1, 3).reshape(P, NK * F))
    maps = []
    for c in range(NCORES):
        th_loc = theta[:, c * BLOC:(c + 1) * BLOC]          # [G, BLOC]
        th_b = np.ascontiguousarray(
            np.broadcast_to(th_loc[None], (P, G, BLOC)).reshape(P, G * BLOC))
        xT = np.ascontiguousarray(
            x[c * BLOC:(c + 1) * BLOC, :].T.reshape(FT, P, BLOC)
            .transpose(1, 0, 2).reshape(P, FT * BLOC))
        maps.append({"A": A_host, "th": th_b, "xT": xT})
    return maps


def _run(theta, x, algebra, order, **kw):
    nc = _build(int(order))
    res = run_bass_kernel_spmd(nc, _in_maps(theta, x, algebra),
                               list(range(NCORES)), **kw)
    out = np.empty((B, F), dtype=np.float32)
    for c in range(NCORES):
        out[c * BLOC:(c + 1) * BLOC, :] = res.results[c]["outT"].T
    return out, res


def kernel(theta, x, algebra, order):
    out, _ = _run(theta, x, algebra, order)
    return out


# revision 12
# speedup vs baseline: 1.0841x; 1.0214x over previous
"""Lie-series expansion kernel for Trainium2 (8 NeuronCores, data-parallel).

result = x + sum_{i=1..order} z_i,  z_i = (1/i) * sum_g diag(theta_g) z_{i-1} A_g

Per step the G=8 generator contraction fuses into ONE [B,4096]x[4096,512]
matmul: stack W_(g,f) = (theta_g/i) * z_{i-1} along the contraction dim.
Data-parallel over batch: each core owns B/8=512 rows, keeps z TRANSPOSED
([feature_partitions, batch_free]) so the theta scaling is a DVE
elementwise op and algebra A[g,f,h] is the stationary operand in natural
layout. Everything lives in SBUF across all steps; float32r matmuls run
at 1 cycle/row (4x over plain fp32).
"""

import numpy as np

import concourse.bass as bass
import concourse.bacc as bacc
import concourse.mybir as mybir
from concourse import tile
from concourse.bass_utils import run_bass_kernel_spmd

G, B, F = 8, 4096, 512
NCORES = 8
BLOC = B // NCORES          # 512 batch rows per core
P = 128                     # partitions
FT = F // P                 # 4 feature tiles
NK = G * FT                 # 32 contraction k-tiles per step
DT = mybir.dt.float32
DTR = mybir.dt.float32r
MULT = mybir.AluOpType.mult

_cache = {}


def _build(order: int):
    if order in _cache:
        return _cache[order]

    nc = bacc.Bacc("TRN2", target_bir_lowering=False, debug=False,
                   num_devices=NCORES)

    A_d = nc.dram_tensor("A", [P, NK * F], DTR, kind="ExternalInput")
    th_d = nc.dram_tensor("th", [P, G * BLOC], DT, kind="ExternalInput")
    xT_d = nc.dram_tensor("xT", [P, FT * BLOC], DT, kind="ExternalInput")
    out_d = nc.dram_tensor("outT", [F, BLOC], DT, kind="ExternalOutput")

    # Consumption order of contraction k-tiles: fi-major (fi outer, g inner).
    korder = [g * FT + fi for fi in range(FT) for g in range(G)]

    with tile.TileContext(nc) as tc:
        with (
            tc.tile_pool(name="const", bufs=1) as cpool,
            tc.tile_pool(name="z", bufs=2) as zpool,
            tc.tile_pool(name="w", bufs=2) as wpool,
            tc.tile_pool(name="psum", bufs=2, space=bass.MemorySpace.PSUM) as ppool,
        ):
            # Two parallel HWDGE rings (sync + scalar), fed round-robin with
            # 256KB chunks in step-1 TRUE dependency order: xT0, th0, A0,
            # th1, A4, ... so the first matmul unblocks after ~3 chunks and
            # the PE chases the stream with 3x supply margin.
            rings = [nc.sync, nc.scalar]
            th = [None] * G
            zT = [None] * FT
            A_t = [None] * NK

            # step 1 runs g-outer (k sequential), so its demand is ~1.25MB
            # per 7.3us g-pass; feed chunks in exactly that order.
            loads = [("x", 0), ("t", 0), ("a", 0), ("a", 1), ("a", 2),
                     ("x", 1), ("x", 2), ("x", 3), ("a", 3)]
            for g in range(1, G):
                loads += [("t", g)] + [("a", g * FT + fi) for fi in range(FT)]

            for n, (kind, j) in enumerate(loads):
                ring = rings[n % 2]
                if kind == "x":
                    zT[j] = zpool.tile([P, BLOC], DT, tag=f"z{j}",
                                       name=f"z{j}_init")
                    ring.dma_start(zT[j][:],
                                   xT_d[:, j * BLOC:(j + 1) * BLOC])
                elif kind == "t":
                    th[j] = cpool.tile([P, BLOC], DT, tag=f"th{j}",
                                       name=f"th{j}")
                    ring.dma_start(th[j][:],
                                   th_d[:, j * BLOC:(j + 1) * BLOC])
                else:
                    A_t[j] = cpool.tile([P, F], DTR, tag=f"A{j}",
                                        name=f"A{j}")
                    ring.dma_start(A_t[j][:], A_d[:, j * F:(j + 1) * F])

            res = []

            def w_build(i, k, src):
                g = k // FT
                w = wpool.tile([P, BLOC], DTR, tag="w", bufs=40,
                               name=f"w_{i}_{k}")
                # w = (theta_g / i) * z_{i-1}
                nc.vector.scalar_tensor_tensor(
                    w[:], th[g][:, :], 1.0 / i, src[:], MULT, MULT)
                return w

            def drain(i, ps_ho, ho, Wn):
                """Consume step i's completed psum bank `ho`: either fold into
                res (last step) or copy to SBUF, accumulate, and build step
                i+1's W tiles for fi=ho."""
                if i == order:
                    nc.vector.scalar_tensor_tensor(
                        res[ho][:], ps_ho[:], 1.0, res[ho][:],
                        MULT, mybir.AluOpType.add)
                    nc.sync.dma_start(out_d[ho * P:(ho + 1) * P, :],
                                      res[ho][:])
                else:
                    zt = zpool.tile([P, BLOC], DT, tag=f"z{ho}",
                                    name=f"z{ho}_{i}")
                    nc.scalar.copy(zt[:], ps_ho[:])
                    nc.vector.tensor_add(res[ho][:], res[ho][:], zt[:])
                    for g in range(G):
                        Wn[g * FT + ho] = w_build(i + 1, g * FT + ho, zt)

            # ---- step 1: g-outer (k sequential), W built inline from x^T
            # (chases the DMA stream with steady ~170GB/s demand) ----
            W = [None] * NK
            ps = [ppool.tile([P, BLOC], DT, tag=f"ps{ho}", name=f"ps{ho}_1")
                  for ho in range(FT)]
            for n, k in enumerate(range(NK)):
                W[k] = w_build(1, k, zT[k % FT])  # k%FT == fi
                for ho in range(FT):
                    nc.tensor.matmul(
                        ps[ho][:], A_t[k][:, ho * P:(ho + 1) * P], W[k][:],
                        start=(n == 0), stop=(n == NK - 1))
            # res = x^T init: emitted AFTER step-1's W builds (DVE is strict
            # FIFO) and routed to the idle GpSimd engine so it never gates
            # the first matmul; only needed by drain(1).
            for ft in range(FT):
                rt = cpool.tile([P, BLOC], DT, tag=f"res{ft}", name=f"res{ft}")
                nc.gpsimd.tensor_copy(rt[:], zT[ft][:])
                res.append(rt)
            Wn = [None] * NK
            for ho in range(FT):
                drain(1, ps[ho], ho, Wn)
            W = Wn

            # ---- steps 2..order: ho-outer so psum banks complete early and
            # step i+1's W tiles pre-build during step i (no boundary bubble)
            for i in range(2, order + 1):
                Wn = [None] * NK
                psn = [ppool.tile([P, BLOC], DT, tag=f"ps{ho}",
                                  name=f"ps{ho}_{i}") for ho in range(FT)]
                for ho in range(FT):
                    for n, k in enumerate(korder):
                        nc.tensor.matmul(
                            psn[ho][:], A_t[k][:, ho * P:(ho + 1) * P], W[k][:],
                            start=(n == 0), stop=(n == NK - 1))
                    drain(i, psn[ho], ho, Wn)
                W = Wn

    nc.compile()
    _cache[order] = nc
    return nc


def _in_maps(theta, x, algebra):
    theta = np.ascontiguousarray(theta, dtype=np.float32)
    x = np.ascontiguousarray(x, dtype=np.float32)
    algebra = np.ascontiguousarray(algebra, dtype=np.float32)
    # A[g, f, h] -> A_host[p, (g*FT+fi)*F + h] with f = fi*128 + p
    A_host = np.ascontiguousarray(
        algebra.reshape(G, FT, P, F).transpose(2, 0, 1, 3).reshape(P, NK * F))
    maps = []
    for c in range(NCORES):
        th_loc = theta[:, c * BLOC:(c + 1) * BLOC]          # [G, BLOC]
        th_b = np.ascontiguousarray(
            np.broadcast_to(th_loc[None], (P, G, BLOC)).reshape(P, G * BLOC))
        xT = np.ascontiguousarray(
            x[c * BLOC:(c + 1) * BLOC, :].T.reshape(FT, P, BLOC)
            .transpose(1, 0, 2).reshape(P, FT * BLOC))
        maps.append({"A": A_host, "th": th_b, "xT": xT})
    return maps


def _run(theta, x, algebra, order, **kw):
    nc = _build(int(order))
    res = run_bass_kernel_spmd(nc, _in_maps(theta, x, algebra),
                               list(range(NCORES)), **kw)
    out = np.empty((B, F), dtype=np.float32)
    for c in range(NCORES):
        out[c * BLOC:(c + 1) * BLOC, :] = res.results[c]["outT"].T
    return out, res


def kernel(theta, x, algebra, order):
    out, _ = _run(theta, x, algebra, order)
    return out


# revision 25
# speedup vs baseline: 1.1405x; 1.0520x over previous
"""Lie-series expansion kernel for Trainium2 (8 NeuronCores, data-parallel).

result = x + sum_{i=1..order} z_i,  z_i = (1/i) * sum_g diag(theta_g) z_{i-1} A_g

Per step the G=8 generator contraction fuses into ONE [B,4096]x[4096,512]
matmul: stack W_(g,f) = (theta_g/i) * z_{i-1} along the contraction dim.
Data-parallel over batch: each core owns B/8=512 rows, keeps z TRANSPOSED
([feature_partitions, batch_free]) so the theta scaling is a DVE
elementwise op and algebra A[g,f,h] is the stationary operand in natural
layout. Everything lives in SBUF across all steps. Matmul operands are
rounded to fp16 (1 cycle/row on the PE, 4x over plain fp32; rel err
~4e-4 end to end, fp32 PSUM accumulation and fp32 state throughout).
"""

import numpy as np

import concourse.bass as bass
import concourse.bacc as bacc
import concourse.mybir as mybir
from concourse import tile
from concourse.bass_utils import run_bass_kernel_spmd

G, B, F = 8, 4096, 512
NCORES = 8
BLOC = B // NCORES          # 512 batch rows per core
P = 128                     # partitions
FT = F // P                 # 4 feature tiles
NK = G * FT                 # 32 contraction k-tiles per step
DT = mybir.dt.float32
DTH = mybir.dt.float16
MULT = mybir.AluOpType.mult

_cache = {}


def _build(order: int):
    if order in _cache:
        return _cache[order]

    nc = bacc.Bacc("TRN2", target_bir_lowering=False, debug=False,
                   num_devices=NCORES)

    A_d = nc.dram_tensor("A", [P, NK * F], DTH, kind="ExternalInput")
    th_d = nc.dram_tensor("th", [P, G * BLOC], DT, kind="ExternalInput")
    xT_d = nc.dram_tensor("xT", [P, FT * BLOC], DT, kind="ExternalInput")
    out_d = nc.dram_tensor("outT", [F, BLOC], DT, kind="ExternalOutput")

    # Consumption order of contraction k-tiles: fi-major (fi outer, g inner).
    korder = [g * FT + fi for fi in range(FT) for g in range(G)]

    with tile.TileContext(nc) as tc:
        with (
            tc.tile_pool(name="const", bufs=1) as cpool,
            tc.tile_pool(name="z", bufs=2) as zpool,
            tc.tile_pool(name="w", bufs=2) as wpool,
            tc.tile_pool(name="psum", bufs=2, space=bass.MemorySpace.PSUM) as ppool,
        ):
            # Two parallel HWDGE rings (sync + scalar), fed round-robin with
            # 256KB chunks in step-1 TRUE dependency order: xT0, th0, A0,
            # th1, A4, ... so the first matmul unblocks after ~3 chunks and
            # the PE chases the stream with 3x supply margin.
            rings = [nc.sync, nc.scalar]
            th = [None] * G
            zT = [None] * FT
            A_t = [None] * NK

            # step 1 runs g-outer (k sequential), so its demand is ~1.25MB
            # per 7.3us g-pass; feed 256KB chunks in exactly that order.
            loads = [("x", 0), ("t", 0), ("a", 0), ("a", 1), ("a", 2),
                     ("x", 1), ("x", 2), ("x", 3), ("a", 3)]
            for g in range(1, G):
                loads += [("t", g)] + [("a", g * FT + fi) for fi in range(FT)]

            for n, (kind, j) in enumerate(loads):
                ring = rings[n % 2]
                if kind == "x":
                    zT[j] = zpool.tile([P, BLOC], DT, tag=f"z{j}",
                                       name=f"z{j}_init")
                    ring.dma_start(zT[j][:],
                                   xT_d[:, j * BLOC:(j + 1) * BLOC])
                elif kind == "t":
                    th[j] = cpool.tile([P, BLOC], DT, tag=f"th{j}",
                                       name=f"th{j}")
                    ring.dma_start(th[j][:],
                                   th_d[:, j * BLOC:(j + 1) * BLOC])
                else:
                    A_t[j] = cpool.tile([P, F], DTH, tag=f"A{j}",
                                        name=f"A{j}")
                    ring.dma_start(A_t[j][:], A_d[:, j * F:(j + 1) * F])

            def A_ap(k, ho):
                return A_t[k][:, ho * P:(ho + 1) * P]

            res = []

            def w_build(i, k, src):
                g = k // FT
                w = wpool.tile([P, BLOC], DTH, tag="w", bufs=40,
                               name=f"w_{i}_{k}")
                # w = (theta_g / i) * z_{i-1}
                nc.vector.scalar_tensor_tensor(
                    w[:], th[g][:, :], 1.0 / i, src[:], MULT, MULT)
                return w

            def drain(i, ps_ho, ho, Wn):
                """Consume step i's completed psum bank `ho`: either fold into
                res (last step) or copy to SBUF, accumulate, and build step
                i+1's W tiles for fi=ho."""
                if i == order:
                    nc.vector.scalar_tensor_tensor(
                        res[ho][:], ps_ho[:], 1.0, res[ho][:],
                        MULT, mybir.AluOpType.add)
                    nc.sync.dma_start(out_d[ho * P:(ho + 1) * P, :],
                                      res[ho][:])
                else:
                    zt = zpool.tile([P, BLOC], DT, tag=f"z{ho}",
                                    name=f"z{ho}_{i}")
                    nc.scalar.copy(zt[:], ps_ho[:])
                    nc.vector.tensor_add(res[ho][:], res[ho][:], zt[:])
                    for g in range(G):
                        Wn[g * FT + ho] = w_build(i + 1, g * FT + ho, zt)

            # ---- step 1: g-outer (k sequential), W built inline from x^T
            # (chases the DMA stream with steady ~170GB/s demand) ----
            W = [None] * NK
            ps = [ppool.tile([P, BLOC], DT, tag=f"ps{ho}", name=f"ps{ho}_1")
                  for ho in range(FT)]
            for n, k in enumerate(range(NK)):
                W[k] = w_build(1, k, zT[k % FT])  # k%FT == fi
                for ho in range(FT):
                    nc.tensor.matmul(
                        ps[ho][:], A_ap(k, ho), W[k][:],
                        start=(n == 0), stop=(n == NK - 1))
            # res = x^T init: emitted AFTER step-1's W builds (DVE is strict
            # FIFO) and routed to the idle GpSimd engine so it never gates
            # the first matmul; only needed by drain(1).
            for ft in range(FT):
                rt = cpool.tile([P, BLOC], DT, tag=f"res{ft}", name=f"res{ft}")
                nc.gpsimd.tensor_copy(rt[:], zT[ft][:])
                res.append(rt)
            Wn = [None] * NK
            for ho in range(FT):
                drain(1, ps[ho], ho, Wn)
            W = Wn

            # ---- steps 2..order: ho-outer so psum banks complete early and
            # step i+1's W tiles pre-build during step i (no boundary bubble)
            for i in range(2, order + 1):
                Wn = [None] * NK
                psn = [ppool.tile([P, BLOC], DT, tag=f"ps{ho}",
                                  name=f"ps{ho}_{i}") for ho in range(FT)]
                for ho in range(FT):
                    for n, k in enumerate(korder):
                        nc.tensor.matmul(
                            psn[ho][:], A_ap(k, ho), W[k][:],
                            start=(n == 0), stop=(n == NK - 1))
                    drain(i, psn[ho], ho, Wn)
                W = Wn

    nc.compile()
    _cache[order] = nc
    return nc


def _in_maps(theta, x, algebra):
    theta = np.ascontiguousarray(theta, dtype=np.float32)
    x = np.ascontiguousarray(x, dtype=np.float32)
    algebra = np.ascontiguousarray(algebra, dtype=np.float32)
    # A[g, f, h] -> A_host[p, (g*FT+fi)*F + h] with f = fi*128 + p
    A_host = np.ascontiguousarray(
        algebra.reshape(G, FT, P, F).transpose(2, 0, 1, 3).reshape(P, NK * F)
        .astype(np.float16))
    maps = []
    for c in range(NCORES):
        th_loc = theta[:, c * BLOC:(c + 1) * BLOC]          # [G, BLOC]
        th_b = np.ascontiguousarray(
            np.broadcast_to(th_loc[None], (P, G, BLOC)).reshape(P, G * BLOC))
        xT = np.ascontiguousarray(
            x[c * BLOC:(c + 1) * BLOC, :].T.reshape(FT, P, BLOC)
            .transpose(1, 0, 2).reshape(P, FT * BLOC))
        maps.append({"A": A_host, "th": th_b, "xT": xT})
    return maps


def _run(theta, x, algebra, order, **kw):
    nc = _build(int(order))
    res = run_bass_kernel_spmd(nc, _in_maps(theta, x, algebra),
                               list(range(NCORES)), **kw)
    out = np.empty((B, F), dtype=np.float32)
    for c in range(NCORES):
        out[c * BLOC:(c + 1) * BLOC, :] = res.results[c]["outT"].T
    return out, res


def kernel(theta, x, algebra, order):
    if int(order) <= 0:
        return np.ascontiguousarray(x, dtype=np.float32).copy()
    out, _ = _run(theta, x, algebra, order)
    return out
